# revision 7
# baseline (speedup 1.0000x reference)
"""AnomalyAttention on 8 Trainium2 NeuronCores (Bass/Tile), data-parallel over batch.

Problem: B,L,H,E = 8,1024,8,64
  score  = (1/sqrt(E)) * einsum('blhe,bshe->bhls', Q, K)
  gauss  = kappa/sig_l * exp(-(l-s)^2 / (2 sig_l^2))       (kappa = 1/sqrt(2 pi))
  G_V    = softmax(score, s) @ G_values
  L_V    = softmax(score + gauss, s) @ L_values

Per core = one batch element, 4 head-pair sweeps in q-major order.
P[s, l] = exp(score) is produced by BOTH ScalarE (ACT exp) and VectorE
(Schraudolph: bf16 bits of exp(x) ~ int16(round((128/ln2)*x + 127*128 - 5.5)),
emitted as one tensor_scalar f32->int16 with bitcast to bf16).  AV work for
half q of sweep i runs interleaved with the next half-sweep's QK/exp steps.
Z = 1^T P via "onescol" matmuls: 4 concurrent col-group streams (M=32 with a
single ones column selecting the output row) into one persistent PSUM bank,
snapshot-drained once per sweep and differenced on the host.
"""

import math
import numpy as np
import ml_dtypes

BF16 = ml_dtypes.bfloat16
B, L, H, E = 8, 1024, 8, 64
NCH = L // 128          # 8 s-chunks of 128
BAND = 16               # gauss band halfwidth (W < 3e-7 beyond; bf16-invisible)
WW = 128 + 2 * BAND     # 192: W tile width in l per s-chunk
N_CORES = 8

SCHR_A = 128.0 / math.log(2.0)          # pre-scaled by 1/sqrt(E) at emit
SCHR_B = 127.0 * 128.0 - 5.5            # round-to-nearest calibrated
VEC_K = {0: (2, 5), 1: (2, 5, 7)}       # k-steps per q-phase exp'd on VectorE

_NC_CACHE = {}


def _build_nc():
    if "nc" in _NC_CACHE:
        return _NC_CACHE["nc"]
    import concourse.bacc as bacc
    import concourse.tile as tile
    from concourse import mybir
    from concourse.tile import add_dep_helper

    f32 = mybir.dt.float32
    bf16 = mybir.dt.bfloat16
    i16 = mybir.dt.int16

    nc = bacc.Bacc()
    qkt_d = nc.declare_dram_parameter("qkt", [4, 128, 2 * L], bf16, isOutput=False)
    # vw[h, :, k, 0:129] = [V_g | V_l | ones];  vw[h, :, k, 129:321] = W band
    vw_d = nc.declare_dram_parameter("vw", [H, 128, NCH, 129 + WW], bf16, isOutput=False)
    outGL_d = nc.declare_dram_parameter("outGL", [H, 2, 128, 512], bf16, isOutput=True)
    outB_d = nc.declare_dram_parameter("outB", [H, 2, 65, 512], bf16, isOutput=True)
    # running Z snapshots (one per sweep); rows 32j+u, u = unit index
    outZ_d = nc.declare_dram_parameter("outZ", [4, 128, 512], f32, isOutput=True)

    with tile.TileContext(nc) as tc:
        with (
            tc.tile_pool(name="const_p", bufs=1) as const_p,
            tc.tile_pool(name="qkt_p", bufs=2) as qkt_p,
            tc.tile_pool(name="vw_p", bufs=4) as vw_p,
            tc.tile_pool(name="pg_p", bufs=26) as pg_p,
            tc.tile_pool(name="mb_p", bufs=30) as mb_p,
            tc.tile_pool(name="stg_p", bufs=8) as stg_p,
            tc.tile_pool(name="sc_p", bufs=2, space="PSUM") as sc_p,
            tc.tile_pool(name="gl_p", bufs=2, space="PSUM") as gl_p,
            tc.tile_pool(name="b_p", bufs=1, space="PSUM") as b_p,
            tc.tile_pool(name="z_p", bufs=1, space="PSUM") as z_p,
        ):
            warm = const_p.tile([128, 512], bf16, tag="warm", bufs=1)
            nc.vector.memset(warm, 0.5)
            zrow = const_p.tile([1, 512], bf16, tag="zrow", bufs=1)
            nc.vector.memset(zrow, 0.0)
            onescol = []
            for u in range(6):
                t = const_p.tile([128, 32], bf16, tag=f"oc{u}", bufs=1)
                nc.vector.memset(t, 0.0)
                nc.vector.memset(t[:, u:u + 1], 1.0)
                onescol.append(t)
            # ACT exp table preload off the critical path
            scr = const_p.tile([128, 16], bf16, tag="scr", bufs=1)
            nc.scalar.activation(out=scr, in_=warm[:, 0:16],
                                 func=mybir.ActivationFunctionType.Exp)

            accZ = z_p.tile([128, 512], f32, tag="accZ", bufs=1)
            # HAM warmup: keep PE busy from t=0 so the clock is at 2.4 GHz
            # when real matmuls arrive (junk results, overwritten below).
            for w in range(7):
                nc.tensor.matmul(out=accZ[0:16, :], lhsT=warm[:, 0:16], rhs=warm,
                                 start=True, stop=True, skip_group_check=True)
            # clear accZ (0s + has_written) for the persistent Z accumulation
            nc.tensor.matmul(out=accZ, lhsT=zrow[:, 0:128], rhs=zrow,
                             start=True, stop=False, skip_group_check=True)

            # ---- DMA helpers --------------------------------------------
            def dma_qkt(i, first=False):
                qt = qkt_p.tile([128, 2 * L], bf16, tag="qkt", bufs=2, name="qt")
                if first:
                    # minimal prefix for QK(q0,k0): K chunk 0 + Q half 0
                    nc.sync.dma_start(out=qt[:, 1024:1152], in_=qkt_d.ap()[i][:, 1024:1152])
                    nc.sync.dma_start(out=qt[:, 0:512], in_=qkt_d.ap()[i][:, 0:512])
                    nc.sync.dma_start(out=qt[:, 1152:2048], in_=qkt_d.ap()[i][:, 1152:2048])
                else:
                    nc.sync.dma_start(out=qt[:, 1024:1536], in_=qkt_d.ap()[i][:, 1024:1536])
                    nc.sync.dma_start(out=qt[:, 0:512], in_=qkt_d.ap()[i][:, 0:512])
                    nc.sync.dma_start(out=qt[:, 1536:2048], in_=qkt_d.ap()[i][:, 1536:2048])
                nc.sync.dma_start(out=qt[:, 512:1024], in_=qkt_d.ap()[i][:, 512:1024])
                return qt

            def dma_vw(h):
                t = vw_p.tile([128, NCH, 129 + WW], bf16, tag="vw", bufs=4, name="vw")
                nc.gpsimd.dma_start(out=t, in_=vw_d.ap()[h])
                return t

            qts = {0: dma_qkt(0, first=True)}
            vws = {0: dma_vw(0), 1: dma_vw(1)}

            # ---- per-sweep state ----------------------------------------
            # pg[(i, q, k)] -> bf16 [128, 1024] P tile (cols 512p per head p)
            pg = {}
            # mb[(i, p, k)] -> bf16 [128, WW] band product tile
            mb = {}

            def emit_qk_exp(i, q, k):
                qt = qts[i]
                sc = sc_p.tile([128, L], f32, tag="sc", bufs=2, name="sc")
                for p in range(2):
                    pslc = slice(64 * p, 64 * p + 64)
                    nc.tensor.matmul(
                        out=sc[:, 512 * p:512 * (p + 1)],
                        lhsT=qt[pslc, L + 128 * k:L + 128 * (k + 1)],
                        rhs=qt[pslc, 512 * q:512 * (q + 1)],
                        start=True, stop=True,
                        tile_position=(64 * p, 0),
                    )
                pgk = pg_p.tile([128, L], bf16, tag="pg", bufs=26, name="pgk")
                if k in VEC_K[q]:
                    nc.vector.tensor_scalar(
                        out=pgk.bitcast(i16), in0=sc,
                        scalar1=SCHR_A / math.sqrt(E), scalar2=SCHR_B,
                        op0=mybir.AluOpType.mult, op1=mybir.AluOpType.add,
                    )
                else:
                    nc.scalar.activation(
                        out=pgk, in_=sc,
                        func=mybir.ActivationFunctionType.Exp,
                        scale=1.0 / math.sqrt(E),
                    )
                pg[(i, q, k)] = pgk

            def emit_mult(i, q, kp):
                """band product slices of mb[i, p, kp] that lie in half q."""
                a0 = max(0, 128 * kp - BAND)
                b0 = min(L, 128 * kp + 128 + BAND)
                a = max(a0, 512 * q)
                bb = min(b0, 512 * (q + 1))
                if bb <= a:
                    return
                for p in range(2):
                    key = (i, p, kp)
                    if key not in mb:
                        mb[key] = mb_p.tile([128, WW], bf16, tag="mb", bufs=30,
                                            name="mbk")
                    woff = a - (128 * kp - BAND)
                    nc.vector.tensor_mul(
                        out=mb[key][:, woff:woff + (bb - a)],
                        in0=pg[(i, q, kp)][:, 512 * p + a - 512 * q:
                                           512 * p + bb - 512 * q],
                        in1=vws[2 * i + p][:, kp, 129 + woff:129 + woff + (bb - a)],
                    )

            def av_unit(i, half, p, u, part):
                """AV work for (head 2i+p, half); part 0..3 slices it."""
                h = 2 * i + p
                vwt = vws[h]
                if part == 0:
                    accGL = gl_p.tile([128, 512], f32, tag="accGL", bufs=2,
                                      name="accGL")
                    av_state[(i, half, p)] = accGL
                    for k in range(4):
                        nc.tensor.matmul(out=accGL, lhsT=vwt[:, k, 0:128],
                                         rhs=pg[(i, half, k)][:, 512 * p:512 * (p + 1)],
                                         start=(k == 0), stop=False)
                elif part == 1:
                    accGL = av_state[(i, half, p)]
                    for k in range(4, NCH):
                        nc.tensor.matmul(out=accGL, lhsT=vwt[:, k, 0:128],
                                         rhs=pg[(i, half, k)][:, 512 * p:512 * (p + 1)],
                                         start=False, stop=(k == NCH - 1))
                    stgGL = stg_p.tile([128, 512], bf16, tag="stgGL", bufs=3,
                                       name="stgGL")
                    nc.scalar.copy(out=stgGL, in_=accGL)
                    nc.sync.dma_start(out=outGL_d.ap()[h, half], in_=stgGL)
                elif part == 2:
                    for k in range(NCH):
                        j = k % 4
                        nc.tensor.matmul(
                            out=accZ[32 * j:32 * j + 32, :],
                            lhsT=onescol[u],
                            rhs=pg[(i, half, k)][:, 512 * p:512 * (p + 1)],
                            start=False, stop=False,
                            tile_position=(0, 32 * j),
                            skip_group_check=True,
                        )
                else:
                    h0 = half * 512
                    spans = []
                    for k in range(NCH):
                        a = max(0, 128 * k - BAND, h0)
                        bb = min(L, 128 * k + 128 + BAND, h0 + 512)
                        if bb > a:
                            spans.append((k, a, bb))
                    accB = b_p.tile([65, 512], f32, tag="accB", bufs=1, name="accB")
                    b_first = None
                    for j, (k, a, bb) in enumerate(spans):
                        off = a - (128 * k - BAND)
                        mmb = nc.tensor.matmul(
                            out=accB[:, a - h0:bb - h0],
                            lhsT=vwt[:, k, 64:129],
                            rhs=mb[(i, p, k)][:, off:off + (bb - a)],
                            start=(j == 0), stop=(j == len(spans) - 1),
                            skip_group_check=True,
                        )
                        if j == 0:
                            b_first = mmb
                        else:
                            add_dep_helper(mmb.ins, b_first.ins,
                                           reason="bank clear first")
                    stgB = stg_p.tile([65, 512], bf16, tag="stgB", bufs=3,
                                      name="stgB")
                    nc.vector.tensor_copy(out=stgB, in_=accB)
                    nc.gpsimd.dma_start(out=outB_d.ap()[h, half], in_=stgB)

            av_state = {}

            def snapshot(s):
                stgZ = stg_p.tile([128, 512], f32, tag="stgZ", bufs=2, name="stgZ")
                nc.vector.tensor_copy(out=stgZ, in_=accZ)
                nc.gpsimd.dma_start(out=outZ_d.ap()[s], in_=stgZ)

            # ---- main loop ----------------------------------------------
            # av-half units processed during sweep i: q0 phase -> (i-1, h1),
            # q1 phase -> (i, h0).  Snapshot u-map: (i,h0):u=p, (i-1,h1):u=2+p,
            # tail (3,h1): u=4+p.
            for i in range(4):
                for q in range(2):
                    av = None
                    if q == 0 and i > 0:
                        av = (i - 1, 1, 2)      # pair i-1, half 1, u-base 2
                    elif q == 1:
                        av = (i, 0, 0)          # pair i,   half 0, u-base 0
                    for k in range(NCH):
                        emit_qk_exp(i, q, k)
                        if k >= 2:
                            emit_mult(i, q, k - 2)
                        if av is not None:
                            ai, ah, ub = av
                            p, part = divmod(k, 4)
                            av_unit(ai, ah, p, ub + p, part)
                        # prefetches during q1 phase
                        if q == 1 and i < 3:
                            if k == 0:
                                qts[i + 1] = dma_qkt(i + 1)
                            elif k == 1:
                                vws[2 * i + 2] = dma_vw(2 * i + 2)
                            elif k == 2:
                                vws[2 * i + 3] = dma_vw(2 * i + 3)
                    emit_mult(i, q, NCH - 2)
                    emit_mult(i, q, NCH - 1)
                # end of sweep i: snapshot (covers av(i,h0) + av(i-1,h1))
                if i < 3:
                    snapshot(i)
            # tail: av(3, h1) then final snapshot
            for p in range(2):
                for part in range(4):
                    av_unit(3, 1, p, 4 + p, part)
            snapshot(3)
    nc.compile()
    _NC_CACHE["nc"] = nc
    return nc


def _host_prep(G_queries, G_keys, G_values, L_values, sigma):
    """Build per-core input dicts + host-side encg [L, H] per core."""
    inv_sqrt_2pi = 1.0 / math.sqrt(2.0 * math.pi)
    sig = sigma.astype(np.float32)
    sig = 1.0 / (1.0 + np.exp(-5.0 * sig.astype(np.float64)))
    sig = (sig + 1e-05).astype(np.float32)
    sig = (np.float32(3.0) ** sig) - np.float32(1.0)          # [B, L, H]
    c = inv_sqrt_2pi / sig.astype(np.float64)                  # [B, L, H]
    encg = np.exp(-c)                                          # [B, L, H]
    nhi = 1.0 / (2.0 * sig.astype(np.float64) ** 2)

    in_maps = []
    aux = []
    for b in range(B):
        qkt = np.empty((4, 128, 2 * L), BF16)
        for h in range(H):
            i, p = divmod(h, 2)
            qkt[i, 64 * p:64 * p + 64, :L] = G_queries[b, :, h, :].T
            qkt[i, 64 * p:64 * p + 64, L:] = G_keys[b, :, h, :].T
        vw = np.zeros((H, 128, NCH, 129 + WW), BF16)
        gv = G_values[b].reshape(NCH, 128, H, E)   # [k, p, h, e]
        lv = L_values[b].reshape(NCH, 128, H, E)
        vw[:, :, :, 0:64] = np.ascontiguousarray(gv.transpose(2, 1, 0, 3))
        vw[:, :, :, 64:128] = np.ascontiguousarray(lv.transpose(2, 1, 0, 3))
        vw[..., 128] = 1.0
        s_off = np.arange(128)
        j_off = np.arange(WW)
        for k in range(NCH):
            s_idx = 128 * k + s_off                  # [128]
            l_idx = 128 * k - BAND + j_off           # [WW]
            valid = (l_idx >= 0) & (l_idx < L)
            lvx = np.clip(l_idx, 0, L - 1)
            d = l_idx[None, :] - s_idx[:, None]      # [128, WW]
            band_ok = (np.abs(d) <= BAND) & valid[None, :]
            for h in range(H):
                ch = c[b, lvx, h][None, :]
                g = ch * np.exp(-(d.astype(np.float64) ** 2) * nhi[b, lvx, h][None, :])
                W = np.exp(g - ch) - encg[b, lvx, h][None, :]
                W[~band_ok] = 0.0
                vw[h, :, k, 129:] = W.astype(np.float32)
        in_maps.append({"qkt": np.asarray(qkt), "vw": np.asarray(vw)})
        aux.append(encg[b])  # [L, H]
    return in_maps, aux


def _host_post(outs, aux):
    G_V = np.empty((B, L, H, E), np.float32)
    L_V = np.empty((B, L, H, E), np.float32)
    jrows = np.arange(4) * 32
    for b in range(B):
        oGL = outs[b]["outGL"].astype(np.float64)  # [H, 2, 128, 512]
        oB = outs[b]["outB"].astype(np.float64)    # [H, 2, 65, 512]
        oZ = outs[b]["outZ"].astype(np.float64)    # [4, 128, 512] running
        dsnap = np.empty_like(oZ)
        dsnap[0] = oZ[0]
        dsnap[1:] = oZ[1:] - oZ[:-1]
        # Z[h=2i+p, half] per u-map
        Z = np.empty((H, 2, 512))
        for i in range(4):
            for p in range(2):
                Z[2 * i + p, 0] = dsnap[i][jrows + p].sum(axis=0)
                if i < 3:
                    Z[2 * i + p, 1] = dsnap[i + 1][jrows + 2 + p].sum(axis=0)
                else:
                    Z[2 * i + p, 1] = dsnap[3][jrows + 4 + p].sum(axis=0)
        for h in range(H):
            GLt = np.concatenate([oGL[h, 0], oGL[h, 1]], axis=1)  # [128, L]
            Bt = np.concatenate([oB[h, 0], oB[h, 1]], axis=1)     # [65, L]
            Zh = np.concatenate([Z[h, 0], Z[h, 1]])               # [L]
            e = aux[b][:, h]  # [L]
            G_V[b, :, h, :] = (GLt[0:64] / Zh).T
            Lnum = GLt[64:128] * e[None, :] + Bt[:64]
            Lden = Zh * e + Bt[64]
            L_V[b, :, h, :] = (Lnum / Lden).T
    return G_V, L_V


def kernel(G_queries, G_keys, G_values, L_values, sigma):
    from concourse.bass_utils import run_bass_kernel_spmd

    args = [np.asarray(x, dtype=np.float32) for x in
            (G_queries, G_keys, G_values, L_values, sigma)]
    nc = _build_nc()
    in_maps, aux = _host_prep(*args)
    res = run_bass_kernel_spmd(nc, in_maps, core_ids=list(range(N_CORES)),
                               trace=False)
    return _host_post(res.results, aux)
